# revision 14
# baseline (speedup 1.0000x reference)
"""Trainium2 Bass kernel for nn_AFM_54022098649750 (dense_mlp).

Reference computation (B=2048, DIM=512, C=64, INTER=128):
    h = relu(bn1(einsum('bdc,cid->bci', x, W1) + b1))
    y = bn2(einsum('bci,cdi->bcd', h, W2) + b2)
    out = sigmoid(transpose(y, (0,2,1)))         # (B, DIM, C)

Strategy:
  * Fold the inference-mode BatchNorms into the conv weights/biases on the
    host (z*s + t == X @ (W*s)^T + (b*s + t)).
  * Branch-parallel sharding: each of the 8 cores owns C_LOC=8 independent
    branches (branch c only consumes x[:, :, c]), so there is no cross-core
    communication and weights are not replicated.
  * Host pre-transposes x so the TensorEngine consumes it directly:
    MM1 computes H^T = W1e @ X^T with the DIM contraction on partitions,
    MM2 consumes H^T in place (INTER contraction on partitions).
  * The work is a uniform stream of 16 units per core (branch x batch-pair):
    1 MB x-chunk DMA in -> 8 accumulating MM1s -> 2 DVE relu(psum+b1) ->
    8 MM2s -> 4 ScalarE sigmoid(psum+b2) over (128,1024) -> 1 MB DMA out.
    Loads ride the SP HWDGE ring, stores the ACT ring, weights the SWDGE
    (GpSimd) path, so the two big streams never queue behind each other.
  * The kernel is DMA-bound: activations stream in/out as bf16 (PSUM
    accumulation and the bias/sigmoid epilogues stay fp32), halving HBM
    traffic.  Set K_MODE=f32r for full-fp32 I/O with reduced-precision
    (tf32-like) matmuls instead.
"""

import os

import ml_dtypes
import numpy as np

import concourse.bacc as bacc
import concourse.bass as bass
import concourse.mybir as mybir
import concourse.tile as tile
from concourse.bass_utils import run_bass_kernel_spmd

B, DIM, C, INTER = 2048, 512, 64, 128
EPS = 1e-5
N_CORES = 8
C_LOC = C // N_CORES          # branches per core
KD = DIM // 128               # contraction / output chunks of MM1 / MM2
NB = 512                      # matmul moving free dim (max with fp32 PSUM out)
BT = B // NB                  # b-tiles per branch
TP = BT // 2                  # b-tile pairs (pipeline unit granularity)
W2NB = 2 * NB                 # unit width in batch elements

MODE = os.environ.get("K_MODE", "bf16")   # "bf16" or "f32r"
XC_BUFS = int(os.environ.get("K_XC", "6"))
OC_BUFS = int(os.environ.get("K_OC", "6"))
W_BUFS = int(os.environ.get("K_W", "2"))
H_BUFS = int(os.environ.get("K_H", "4"))
N_XACT = int(os.environ.get("K_XACT", "4"))   # leading x-chunks on ACT ring

F32 = mybir.dt.float32
AFT = mybir.ActivationFunctionType

_CACHE = {}


def _io_dt():
    return mybir.dt.bfloat16 if MODE == "bf16" else mybir.dt.float32r


def _io_np():
    return ml_dtypes.bfloat16 if MODE == "bf16" else np.float32


def _build():
    """Build + compile the per-core Bass graph (same graph on all cores)."""
    IO = _io_dt()
    nc = bacc.Bacc("TRN2", target_bir_lowering=False, debug=False,
                   num_devices=N_CORES)

    xt = nc.dram_tensor("xt", [C_LOC, TP, KD, 128, W2NB], IO,
                        kind="ExternalInput").ap()
    w1t = nc.dram_tensor("w1t", [C_LOC, DIM, INTER], IO,
                         kind="ExternalInput").ap()
    w2t = nc.dram_tensor("w2t", [C_LOC, INTER, DIM], IO,
                         kind="ExternalInput").ap()
    bt = nc.dram_tensor("bt", [128, (KD + 1) * C_LOC], F32,
                        kind="ExternalInput").ap()
    out = nc.dram_tensor("out", [C_LOC, TP, KD, 128, W2NB], IO,
                         kind="ExternalOutput").ap()

    with tile.TileContext(nc) as tc:
        with (
            tc.tile_pool(name="xcp", bufs=XC_BUFS) as xcp,
            tc.tile_pool(name="ocp", bufs=OC_BUFS) as ocp,
            tc.tile_pool(name="w1p", bufs=W_BUFS) as w1p,
            tc.tile_pool(name="w2p", bufs=W_BUFS) as w2p,
            tc.tile_pool(name="hp", bufs=H_BUFS) as hp,
            tc.tile_pool(name="bp", bufs=1) as bp,
            tc.tile_pool(name="php", bufs=2, space="PSUM") as php,
            tc.tile_pool(name="pyp", bufs=3, space="PSUM") as pyp,
        ):
            bs = bp.tile([128, (KD + 1) * C_LOC], F32, tag="bs")
            nc.gpsimd.dma_start(bs[:], bt[:])
            b1 = bs[:, 0:C_LOC]
            b2 = bs[:, C_LOC:]

            unit = 0
            for c in range(C_LOC):
                # branch 0's weights first on the SP ring so the first
                # matmul isn't gated on the slower SWDGE path; steady
                # state weights go via GpSimd (SWDGE) off both HW rings.
                weng = nc.sync if c == 0 else nc.gpsimd
                w1 = w1p.tile([128, KD * INTER], IO, tag="w1")
                weng.dma_start(
                    w1[:].rearrange("p (k i) -> p k i", k=KD),
                    w1t[c].rearrange("(k p) i -> p k i", p=128),
                )
                w2 = w2p.tile([INTER, DIM], IO, tag="w2")
                weng.dma_start(w2[:], w2t[c])

                for tp_ in range(TP):
                    # 1 MB x chunk: partitions = d-in-chunk, free =
                    # (k, b-pair).  The ACT HWDGE ring is idle until the
                    # first sigmoids; ramp the pipeline through it.
                    xeng = nc.scalar if unit < N_XACT else nc.sync
                    xc = xcp.tile([128, KD * W2NB], IO, tag="xc",
                                  name=f"xc{c}_{tp_}")
                    xeng.dma_start(
                        xc[:].rearrange("p (k b) -> p k b", k=KD),
                        xt[c, tp_].rearrange("k p b -> p k b"),
                    )
                    hs = []
                    for j in range(2):
                        ph = php.tile([INTER, NB], F32, tag="ph")
                        for k in range(KD):
                            nc.tensor.matmul(
                                ph[:],
                                w1[:, k * INTER:(k + 1) * INTER],
                                xc[:, k * W2NB + j * NB:
                                   k * W2NB + (j + 1) * NB],
                                start=(k == 0),
                                stop=(k == KD - 1),
                            )
                        h = hp.tile([INTER, NB], IO, tag="h",
                                    name=f"h{c}_{tp_}_{j}")
                        if MODE == "bf16":
                            # relu(psum + b1) on the Vector engine
                            # (keeps ScalarE free for the sigmoids)
                            nc.vector.tensor_scalar(
                                h[:], ph[:], b1[:, c:c + 1], 0.0,
                                mybir.AluOpType.add, mybir.AluOpType.max,
                            )
                        else:
                            nc.scalar.activation(h[:], ph[:], AFT.Relu,
                                                 bias=b1[:, c:c + 1])
                        hs.append(h)

                    oc = ocp.tile([128, KD * W2NB], IO, tag="oc",
                                  name=f"oc{c}_{tp_}")
                    for k in range(KD):
                        py = pyp.tile([128, W2NB], F32, tag="py")
                        for j in range(2):
                            nc.tensor.matmul(
                                py[:, j * NB:(j + 1) * NB],
                                w2[:, k * 128:(k + 1) * 128],
                                hs[j][:],
                                start=True,
                                stop=True,
                            )
                        nc.scalar.activation(
                            oc[:, k * W2NB:(k + 1) * W2NB],
                            py[:], AFT.Sigmoid,
                            bias=b2[:, k * C_LOC + c: k * C_LOC + c + 1],
                        )
                    # 1 MB contiguous store on the ACT ring
                    nc.scalar.dma_start(
                        out[c, tp_].rearrange("k p b -> p k b"),
                        oc[:].rearrange("p (k b) -> p k b", k=KD),
                    )
                    unit += 1

    nc.compile()
    return nc


def _prep_in_maps(x, W1, b1, g1, be1, m1, v1, W2, b2, g2, be2, m2, v2):
    """Fold BN, transpose to device layouts, slice per-core shards."""
    io_np = _io_np()
    s1 = (g1 / np.sqrt(v1 + EPS)).astype(np.float32)          # (C, INTER)
    b1e = (b1 * s1 + be1 - m1 * s1).astype(np.float32)        # (C, INTER)
    s2 = (g2 / np.sqrt(v2 + EPS)).astype(np.float32)          # (C, DIM)
    b2e = (b2 * s2 + be2 - m2 * s2).astype(np.float32)        # (C, DIM)

    w1t = np.ascontiguousarray(
        (W1 * s1[:, :, None]).transpose(0, 2, 1)).astype(io_np)  # (C, DIM, INTER)
    w2t = np.ascontiguousarray(
        (W2 * s2[:, :, None]).transpose(0, 2, 1)).astype(io_np)  # (C, INTER, DIM)
    # x (B, DIM, C) -> (C, TP, KD, 128, W2NB):
    #   [c, tp, k, p, col] = x[tp*W2NB + col, k*128 + p, c]
    xv = np.asarray(x).astype(io_np).reshape(TP, W2NB, KD, 128, C)
    xt = np.ascontiguousarray(xv.transpose(4, 0, 2, 3, 1))
    b1tt = np.ascontiguousarray(b1e.T)                        # (INTER, C)
    # (128, KD, C): bias for output chunk k, partition d_in, branch c
    b2tt = np.ascontiguousarray(
        b2e.reshape(C, KD, 128).transpose(2, 1, 0))

    in_maps = []
    for m in range(N_CORES):
        lo, hi = m * C_LOC, (m + 1) * C_LOC
        in_maps.append({
            "xt": np.ascontiguousarray(xt[lo:hi]),
            "w1t": np.ascontiguousarray(w1t[lo:hi]),
            "w2t": np.ascontiguousarray(w2t[lo:hi]),
            "bt": np.concatenate([
                np.ascontiguousarray(b1tt[:, lo:hi]),
                np.ascontiguousarray(
                    b2tt[:, :, lo:hi]).reshape(128, KD * C_LOC),
            ], axis=1),
        })
    return in_maps


def _unshard(results):
    """(C_LOC, TP, KD, 128, W2NB) per core -> (B, DIM, C)."""
    full = np.empty((B, DIM, C), dtype=np.float32)
    for m in range(N_CORES):
        shard = np.asarray(results[m]["out"]).astype(np.float32)
        # [c, tp, k, p, col] -> out[tp*W2NB+col, k*128+p, c]
        full[:, :, m * C_LOC:(m + 1) * C_LOC] = (
            shard.transpose(1, 4, 2, 3, 0).reshape(B, DIM, C_LOC))
    return full


def _run(in_maps, trace=False, tmpdir=None):
    if "nc" not in _CACHE:
        _CACHE["nc"] = _build()
    return run_bass_kernel_spmd(
        _CACHE["nc"], in_maps, core_ids=list(range(N_CORES)),
        trace=trace, tmpdir=tmpdir)


def kernel(**inputs):
    in_maps = _prep_in_maps(**inputs)
    res = _run(in_maps)
    return _unshard(res.results)


def kernel_with_profile(tmpdir=None, **inputs):
    """Like kernel() but also returns neuron-profile exec_time_ns."""
    in_maps = _prep_in_maps(**inputs)
    res = _run(in_maps, trace=True, tmpdir=tmpdir)
    return _unshard(res.results), res.exec_time_ns


# revision 17
# speedup vs baseline: 1.0638x; 1.0638x over previous
"""Trainium2 Bass kernel for nn_AFM_54022098649750 (dense_mlp).

Reference computation (B=2048, DIM=512, C=64, INTER=128):
    h = relu(bn1(einsum('bdc,cid->bci', x, W1) + b1))
    y = bn2(einsum('bci,cdi->bcd', h, W2) + b2)
    out = sigmoid(transpose(y, (0,2,1)))         # (B, DIM, C)

Strategy:
  * Fold the inference-mode BatchNorms into the conv weights/biases on the
    host (z*s + t == X @ (W*s)^T + (b*s + t)).
  * Branch-parallel sharding: each of the 8 cores owns C_LOC=8 independent
    branches (branch c only consumes x[:, :, c]), so there is no cross-core
    communication and weights are not replicated.
  * Host pre-transposes x so the TensorEngine consumes it directly:
    MM1 computes H^T = W1e @ X^T with the DIM contraction on partitions,
    MM2 consumes H^T in place (INTER contraction on partitions).
  * The work is a uniform stream of 16 units per core (branch x batch-pair):
    1 MB x-chunk DMA in -> 8 accumulating MM1s -> 2 DVE relu(psum+b1) ->
    8 MM2s -> 4 ScalarE sigmoid(psum+b2) over (128,1024) -> 1 MB DMA out.
    Loads ride the SP HWDGE ring, stores the ACT ring, weights the SWDGE
    (GpSimd) path, so the two big streams never queue behind each other.
  * The kernel is DMA-bound: activations stream in/out as bf16 (PSUM
    accumulation and the bias/sigmoid epilogues stay fp32), halving HBM
    traffic.  Set K_MODE=f32r for full-fp32 I/O with reduced-precision
    (tf32-like) matmuls instead.
"""

import os

import ml_dtypes
import numpy as np

import concourse.bacc as bacc
import concourse.bass as bass
import concourse.mybir as mybir
import concourse.tile as tile
from concourse.bass_utils import run_bass_kernel_spmd

B, DIM, C, INTER = 2048, 512, 64, 128
EPS = 1e-5
N_CORES = 8
C_LOC = C // N_CORES          # branches per core
KD = DIM // 128               # contraction / output chunks of MM1 / MM2
NB = 512                      # matmul moving free dim (max with fp32 PSUM out)
BT = B // NB                  # b-tiles per branch
TP = int(os.environ.get("K_TP", "2"))   # pipeline units per branch
W2NB = B // TP                # unit width in batch elements
JW = W2NB // NB               # NB-wide b-tiles per unit

MODE = os.environ.get("K_MODE", "bf16")   # "bf16" or "f32r"
XC_BUFS = int(os.environ.get("K_XC", "6"))
OC_BUFS = int(os.environ.get("K_OC", "6"))
W_BUFS = int(os.environ.get("K_W", "2"))
H_BUFS = int(os.environ.get("K_H", "4"))
N_XACT = int(os.environ.get("K_XACT", "2"))   # leading x-chunks on ACT ring

F32 = mybir.dt.float32
AFT = mybir.ActivationFunctionType

_CACHE = {}


def _io_dt():
    return mybir.dt.bfloat16 if MODE == "bf16" else mybir.dt.float32r


def _io_np():
    return ml_dtypes.bfloat16 if MODE == "bf16" else np.float32


def _build(xc_bufs=None, oc_bufs=None, w_bufs=None, h_bufs=None,
           n_xact=None):
    """Build + compile the per-core Bass graph (same graph on all cores)."""
    xc_bufs = XC_BUFS if xc_bufs is None else xc_bufs
    oc_bufs = OC_BUFS if oc_bufs is None else oc_bufs
    w_bufs = W_BUFS if w_bufs is None else w_bufs
    h_bufs = H_BUFS if h_bufs is None else h_bufs
    n_xact = N_XACT if n_xact is None else n_xact
    IO = _io_dt()
    nc = bacc.Bacc("TRN2", target_bir_lowering=False, debug=False,
                   num_devices=N_CORES)

    xt = nc.dram_tensor("xt", [C_LOC, TP, KD, 128, W2NB], IO,
                        kind="ExternalInput").ap()
    w1t = nc.dram_tensor("w1t", [C_LOC, DIM, INTER], IO,
                         kind="ExternalInput").ap()
    w2t = nc.dram_tensor("w2t", [C_LOC, INTER, DIM], IO,
                         kind="ExternalInput").ap()
    bt = nc.dram_tensor("bt", [128, (KD + 1) * C_LOC], F32,
                        kind="ExternalInput").ap()
    out = nc.dram_tensor("out", [C_LOC, TP, KD, 128, W2NB], IO,
                         kind="ExternalOutput").ap()

    with tile.TileContext(nc) as tc:
        with (
            tc.tile_pool(name="xcp", bufs=xc_bufs) as xcp,
            tc.tile_pool(name="ocp", bufs=oc_bufs) as ocp,
            tc.tile_pool(name="w1p", bufs=w_bufs) as w1p,
            tc.tile_pool(name="w2p", bufs=w_bufs) as w2p,
            tc.tile_pool(name="hp", bufs=h_bufs) as hp,
            tc.tile_pool(name="bp", bufs=1) as bp,
            tc.tile_pool(name="php", bufs=2, space="PSUM") as php,
            tc.tile_pool(name="pyp", bufs=3, space="PSUM") as pyp,
        ):
            bs = bp.tile([128, (KD + 1) * C_LOC], F32, tag="bs")
            nc.gpsimd.dma_start(bs[:], bt[:])
            b1 = bs[:, 0:C_LOC]
            b2 = bs[:, C_LOC:]

            unit = 0
            for c in range(C_LOC):
                # branch 0's weights first on the SP ring so the first
                # matmul isn't gated on the slower SWDGE path; steady
                # state weights go via GpSimd (SWDGE) off both HW rings.
                weng = nc.sync if c == 0 else nc.gpsimd
                w1 = w1p.tile([128, KD * INTER], IO, tag="w1")
                weng.dma_start(
                    w1[:].rearrange("p (k i) -> p k i", k=KD),
                    w1t[c].rearrange("(k p) i -> p k i", p=128),
                )
                w2 = w2p.tile([INTER, DIM], IO, tag="w2")
                weng.dma_start(w2[:], w2t[c])

                for tp_ in range(TP):
                    # 1 MB x chunk: partitions = d-in-chunk, free =
                    # (k, b-pair).  The ACT HWDGE ring is idle until the
                    # first sigmoids; ramp the pipeline through it.
                    xeng = nc.scalar if unit < n_xact else nc.sync
                    xc = xcp.tile([128, KD * W2NB], IO, tag="xc",
                                  name=f"xc{c}_{tp_}")
                    xeng.dma_start(
                        xc[:].rearrange("p (k b) -> p k b", k=KD),
                        xt[c, tp_].rearrange("k p b -> p k b"),
                    )
                    hs = []
                    for j in range(JW):
                        ph = php.tile([INTER, NB], F32, tag="ph")
                        for k in range(KD):
                            nc.tensor.matmul(
                                ph[:],
                                w1[:, k * INTER:(k + 1) * INTER],
                                xc[:, k * W2NB + j * NB:
                                   k * W2NB + (j + 1) * NB],
                                start=(k == 0),
                                stop=(k == KD - 1),
                            )
                        h = hp.tile([INTER, NB], IO, tag="h",
                                    name=f"h{c}_{tp_}_{j}")
                        if MODE == "bf16":
                            # relu(psum + b1) on the Vector engine
                            # (keeps ScalarE free for the sigmoids)
                            nc.vector.tensor_scalar(
                                h[:], ph[:], b1[:, c:c + 1], 0.0,
                                mybir.AluOpType.add, mybir.AluOpType.max,
                            )
                        else:
                            nc.scalar.activation(h[:], ph[:], AFT.Relu,
                                                 bias=b1[:, c:c + 1])
                        hs.append(h)

                    oc = ocp.tile([128, KD * W2NB], IO, tag="oc",
                                  name=f"oc{c}_{tp_}")
                    for k in range(KD):
                        for pr in range(JW // 2):
                            py = pyp.tile([128, 2 * NB], F32, tag="py")
                            for j2 in range(2):
                                j = 2 * pr + j2
                                nc.tensor.matmul(
                                    py[:, j2 * NB:(j2 + 1) * NB],
                                    w2[:, k * 128:(k + 1) * 128],
                                    hs[j][:],
                                    start=True,
                                    stop=True,
                                )
                            nc.scalar.activation(
                                oc[:, k * W2NB + 2 * pr * NB:
                                   k * W2NB + 2 * (pr + 1) * NB],
                                py[:], AFT.Sigmoid,
                                bias=b2[:, k * C_LOC + c:
                                        k * C_LOC + c + 1],
                            )
                    # 1 MB contiguous store on the ACT ring
                    nc.scalar.dma_start(
                        out[c, tp_].rearrange("k p b -> p k b"),
                        oc[:].rearrange("p (k b) -> p k b", k=KD),
                    )
                    unit += 1

    nc.compile()
    return nc


def _prep_in_maps(x, W1, b1, g1, be1, m1, v1, W2, b2, g2, be2, m2, v2):
    """Fold BN, transpose to device layouts, slice per-core shards."""
    x, W1, b1, g1, be1, m1, v1, W2, b2, g2, be2, m2, v2 = (
        np.asarray(a, dtype=np.float32)
        for a in (x, W1, b1, g1, be1, m1, v1, W2, b2, g2, be2, m2, v2))
    io_np = _io_np()
    s1 = (g1 / np.sqrt(v1 + EPS)).astype(np.float32)          # (C, INTER)
    b1e = (b1 * s1 + be1 - m1 * s1).astype(np.float32)        # (C, INTER)
    s2 = (g2 / np.sqrt(v2 + EPS)).astype(np.float32)          # (C, DIM)
    b2e = (b2 * s2 + be2 - m2 * s2).astype(np.float32)        # (C, DIM)

    w1t = np.ascontiguousarray(
        (W1 * s1[:, :, None]).transpose(0, 2, 1)).astype(io_np)  # (C, DIM, INTER)
    w2t = np.ascontiguousarray(
        (W2 * s2[:, :, None]).transpose(0, 2, 1)).astype(io_np)  # (C, INTER, DIM)
    # x (B, DIM, C) -> (C, TP, KD, 128, W2NB):
    #   [c, tp, k, p, col] = x[tp*W2NB + col, k*128 + p, c]
    xv = x.astype(io_np).reshape(TP, W2NB, KD, 128, C)
    xt = np.ascontiguousarray(xv.transpose(4, 0, 2, 3, 1))
    b1tt = np.ascontiguousarray(b1e.T)                        # (INTER, C)
    # (128, KD, C): bias for output chunk k, partition d_in, branch c
    b2tt = np.ascontiguousarray(
        b2e.reshape(C, KD, 128).transpose(2, 1, 0))

    in_maps = []
    for m in range(N_CORES):
        lo, hi = m * C_LOC, (m + 1) * C_LOC
        in_maps.append({
            "xt": np.ascontiguousarray(xt[lo:hi]),
            "w1t": np.ascontiguousarray(w1t[lo:hi]),
            "w2t": np.ascontiguousarray(w2t[lo:hi]),
            "bt": np.concatenate([
                np.ascontiguousarray(b1tt[:, lo:hi]),
                np.ascontiguousarray(
                    b2tt[:, :, lo:hi]).reshape(128, KD * C_LOC),
            ], axis=1),
        })
    return in_maps


def _unshard(results):
    """(C_LOC, TP, KD, 128, W2NB) per core -> (B, DIM, C)."""
    full = np.empty((B, DIM, C), dtype=np.float32)
    for m in range(N_CORES):
        shard = np.asarray(results[m]["out"]).astype(np.float32)
        # [c, tp, k, p, col] -> out[tp*W2NB+col, k*128+p, c]
        full[:, :, m * C_LOC:(m + 1) * C_LOC] = (
            shard.transpose(1, 4, 2, 3, 0).reshape(B, DIM, C_LOC))
    return full


def _run(in_maps, trace=False, tmpdir=None):
    if "nc" not in _CACHE:
        _CACHE["nc"] = _build()
    return run_bass_kernel_spmd(
        _CACHE["nc"], in_maps, core_ids=list(range(N_CORES)),
        trace=trace, tmpdir=tmpdir)


def kernel(**inputs):
    in_maps = _prep_in_maps(**inputs)
    res = _run(in_maps)
    return _unshard(res.results)


def kernel_with_profile(tmpdir=None, **inputs):
    """Like kernel() but also returns neuron-profile exec_time_ns."""
    in_maps = _prep_in_maps(**inputs)
    res = _run(in_maps, trace=True, tmpdir=tmpdir)
    return _unshard(res.results), res.exec_time_ns


# revision 18
# speedup vs baseline: 1.1875x; 1.1162x over previous
"""Trainium2 Bass kernel for nn_AFM_54022098649750 (dense_mlp).

Reference computation (B=2048, DIM=512, C=64, INTER=128):
    h = relu(bn1(einsum('bdc,cid->bci', x, W1) + b1))
    y = bn2(einsum('bci,cdi->bcd', h, W2) + b2)
    out = sigmoid(transpose(y, (0,2,1)))         # (B, DIM, C)

Strategy:
  * Fold the inference-mode BatchNorms into the conv weights/biases on the
    host (z*s + t == X @ (W*s)^T + (b*s + t)).
  * Branch-parallel sharding: each of the 8 cores owns C_LOC=8 independent
    branches (branch c only consumes x[:, :, c]), so there is no cross-core
    communication and weights are not replicated.
  * Host pre-transposes x so the TensorEngine consumes it directly:
    MM1 computes H^T = W1e @ X^T with the DIM contraction on partitions,
    MM2 consumes H^T in place (INTER contraction on partitions).
  * The work is a uniform stream of 16 units per core (branch x batch-pair):
    1 MB x-chunk DMA in -> 8 accumulating MM1s -> 2 DVE relu(psum+b1) ->
    8 MM2s -> 4 ScalarE sigmoid(psum+b2) over (128,1024) -> 1 MB DMA out.
    Loads ride the SP HWDGE ring, stores the ACT ring, weights the SWDGE
    (GpSimd) path, so the two big streams never queue behind each other.
  * The kernel is DMA-bound: activations stream in/out as bf16 (PSUM
    accumulation and the bias/sigmoid epilogues stay fp32), halving HBM
    traffic.  Set K_MODE=f32r for full-fp32 I/O with reduced-precision
    (tf32-like) matmuls instead.
"""

import os

import ml_dtypes
import numpy as np

import concourse.bacc as bacc
import concourse.bass as bass
import concourse.mybir as mybir
import concourse.tile as tile
from concourse.bass_utils import run_bass_kernel_spmd

B, DIM, C, INTER = 2048, 512, 64, 128
EPS = 1e-5
N_CORES = 8
C_LOC = C // N_CORES          # branches per core
KD = DIM // 128               # contraction / output chunks of MM1 / MM2
NB = 512                      # matmul moving free dim (max with fp32 PSUM out)
BT = B // NB                  # b-tiles per branch
TP = int(os.environ.get("K_TP", "2"))   # pipeline units per branch
W2NB = B // TP                # unit width in batch elements
JW = W2NB // NB               # NB-wide b-tiles per unit

MODE = os.environ.get("K_MODE", "bf16")   # "bf16" or "f32r"
XC_BUFS = int(os.environ.get("K_XC", "6"))
OC_BUFS = int(os.environ.get("K_OC", "6"))
W_BUFS = int(os.environ.get("K_W", "2"))
H_BUFS = int(os.environ.get("K_H", "4"))
N_XACT = int(os.environ.get("K_XACT", "2"))   # leading x-chunks on ACT ring

F32 = mybir.dt.float32
AFT = mybir.ActivationFunctionType

_CACHE = {}


def _io_dt():
    return mybir.dt.bfloat16 if MODE == "bf16" else mybir.dt.float32r


def _io_np():
    return ml_dtypes.bfloat16 if MODE == "bf16" else np.float32


def _build(xc_bufs=None, oc_bufs=None, w_bufs=None, h_bufs=None,
           n_xact=None):
    """Build + compile the per-core Bass graph (same graph on all cores)."""
    xc_bufs = XC_BUFS if xc_bufs is None else xc_bufs
    oc_bufs = OC_BUFS if oc_bufs is None else oc_bufs
    w_bufs = W_BUFS if w_bufs is None else w_bufs
    h_bufs = H_BUFS if h_bufs is None else h_bufs
    n_xact = N_XACT if n_xact is None else n_xact
    IO = _io_dt()
    nc = bacc.Bacc("TRN2", target_bir_lowering=False, debug=False,
                   num_devices=N_CORES)

    xt = nc.dram_tensor("xt", [C_LOC, TP, 128, KD * W2NB], IO,
                        kind="ExternalInput").ap()
    w1t = nc.dram_tensor("w1t", [C_LOC, 128, KD * INTER], IO,
                         kind="ExternalInput").ap()
    w2t = nc.dram_tensor("w2t", [C_LOC, INTER, DIM], IO,
                         kind="ExternalInput").ap()
    bt = nc.dram_tensor("bt", [128, (KD + 1) * C_LOC], F32,
                        kind="ExternalInput").ap()
    out = nc.dram_tensor("out", [C_LOC, TP, 128, KD * W2NB], IO,
                         kind="ExternalOutput").ap()

    with tile.TileContext(nc) as tc:
        with (
            tc.tile_pool(name="xcp", bufs=xc_bufs) as xcp,
            tc.tile_pool(name="ocp", bufs=oc_bufs) as ocp,
            tc.tile_pool(name="w1p", bufs=w_bufs) as w1p,
            tc.tile_pool(name="w2p", bufs=w_bufs) as w2p,
            tc.tile_pool(name="hp", bufs=h_bufs) as hp,
            tc.tile_pool(name="bp", bufs=1) as bp,
            tc.tile_pool(name="php", bufs=2, space="PSUM") as php,
            tc.tile_pool(name="pyp", bufs=3, space="PSUM") as pyp,
        ):
            bs = bp.tile([128, (KD + 1) * C_LOC], F32, tag="bs")
            nc.gpsimd.dma_start(bs[:], bt[:])
            b1 = bs[:, 0:C_LOC]
            b2 = bs[:, C_LOC:]

            unit = 0
            for c in range(C_LOC):
                # branch 0's weights first on the SP ring so the first
                # matmul isn't gated on the slower SWDGE path; steady
                # state weights go via GpSimd (SWDGE) off both HW rings.
                weng = nc.sync if c == 0 else nc.gpsimd
                w1 = w1p.tile([128, KD * INTER], IO, tag="w1")
                weng.dma_start(w1[:], w1t[c])
                w2 = w2p.tile([INTER, DIM], IO, tag="w2")
                weng.dma_start(w2[:], w2t[c])

                for tp_ in range(TP):
                    # 1 MB x chunk: partitions = d-in-chunk, free =
                    # (k, b-pair).  The ACT HWDGE ring is idle until the
                    # first sigmoids; ramp the pipeline through it.
                    xeng = nc.scalar if unit < n_xact else nc.sync
                    xc = xcp.tile([128, KD * W2NB], IO, tag="xc",
                                  name=f"xc{c}_{tp_}")
                    xeng.dma_start(xc[:], xt[c, tp_])
                    hs = []
                    for j in range(JW):
                        ph = php.tile([INTER, NB], F32, tag="ph")
                        for k in range(KD):
                            nc.tensor.matmul(
                                ph[:],
                                w1[:, k * INTER:(k + 1) * INTER],
                                xc[:, k * W2NB + j * NB:
                                   k * W2NB + (j + 1) * NB],
                                start=(k == 0),
                                stop=(k == KD - 1),
                            )
                        h = hp.tile([INTER, NB], IO, tag="h",
                                    name=f"h{c}_{tp_}_{j}")
                        if MODE == "bf16":
                            # relu(psum + b1) on the Vector engine
                            # (keeps ScalarE free for the sigmoids)
                            nc.vector.tensor_scalar(
                                h[:], ph[:], b1[:, c:c + 1], 0.0,
                                mybir.AluOpType.add, mybir.AluOpType.max,
                            )
                        else:
                            nc.scalar.activation(h[:], ph[:], AFT.Relu,
                                                 bias=b1[:, c:c + 1])
                        hs.append(h)

                    oc = ocp.tile([128, KD * W2NB], IO, tag="oc",
                                  name=f"oc{c}_{tp_}")
                    for k in range(KD):
                        for pr in range(JW // 2):
                            py = pyp.tile([128, 2 * NB], F32, tag="py")
                            for j2 in range(2):
                                j = 2 * pr + j2
                                nc.tensor.matmul(
                                    py[:, j2 * NB:(j2 + 1) * NB],
                                    w2[:, k * 128:(k + 1) * 128],
                                    hs[j][:],
                                    start=True,
                                    stop=True,
                                )
                            nc.scalar.activation(
                                oc[:, k * W2NB + 2 * pr * NB:
                                   k * W2NB + 2 * (pr + 1) * NB],
                                py[:], AFT.Sigmoid,
                                bias=b2[:, k * C_LOC + c:
                                        k * C_LOC + c + 1],
                            )
                    # 1 MB contiguous store on the ACT ring
                    nc.scalar.dma_start(out[c, tp_], oc[:])
                    unit += 1

    nc.compile()
    return nc


def _prep_in_maps(x, W1, b1, g1, be1, m1, v1, W2, b2, g2, be2, m2, v2):
    """Fold BN, transpose to device layouts, slice per-core shards."""
    x, W1, b1, g1, be1, m1, v1, W2, b2, g2, be2, m2, v2 = (
        np.asarray(a, dtype=np.float32)
        for a in (x, W1, b1, g1, be1, m1, v1, W2, b2, g2, be2, m2, v2))
    io_np = _io_np()
    s1 = (g1 / np.sqrt(v1 + EPS)).astype(np.float32)          # (C, INTER)
    b1e = (b1 * s1 + be1 - m1 * s1).astype(np.float32)        # (C, INTER)
    s2 = (g2 / np.sqrt(v2 + EPS)).astype(np.float32)          # (C, DIM)
    b2e = (b2 * s2 + be2 - m2 * s2).astype(np.float32)        # (C, DIM)

    # (C, 128, KD*INTER): per-partition-contiguous stationary layout
    w1t = np.ascontiguousarray(
        (W1 * s1[:, :, None]).transpose(0, 2, 1).reshape(C, KD, 128, INTER)
        .transpose(0, 2, 1, 3).reshape(C, 128, KD * INTER)).astype(io_np)
    w2t = np.ascontiguousarray(
        (W2 * s2[:, :, None]).transpose(0, 2, 1)).astype(io_np)  # (C, INTER, DIM)
    # x (B, DIM, C) -> (C, TP, KD, 128, W2NB):
    #   [c, tp, k, p, col] = x[tp*W2NB + col, k*128 + p, c]
    xv = x.astype(io_np).reshape(TP, W2NB, KD, 128, C)
    xt = np.ascontiguousarray(
        xv.transpose(4, 0, 3, 2, 1).reshape(C, TP, 128, KD * W2NB))
    b1tt = np.ascontiguousarray(b1e.T)                        # (INTER, C)
    # (128, KD, C): bias for output chunk k, partition d_in, branch c
    b2tt = np.ascontiguousarray(
        b2e.reshape(C, KD, 128).transpose(2, 1, 0))

    in_maps = []
    for m in range(N_CORES):
        lo, hi = m * C_LOC, (m + 1) * C_LOC
        in_maps.append({
            "xt": np.ascontiguousarray(xt[lo:hi]),
            "w1t": np.ascontiguousarray(w1t[lo:hi]),
            "w2t": np.ascontiguousarray(w2t[lo:hi]),
            "bt": np.concatenate([
                np.ascontiguousarray(b1tt[:, lo:hi]),
                np.ascontiguousarray(
                    b2tt[:, :, lo:hi]).reshape(128, KD * C_LOC),
            ], axis=1),
        })
    return in_maps


def _unshard(results):
    """(C_LOC, TP, 128, KD*W2NB) per core -> (B, DIM, C)."""
    full = np.empty((B, DIM, C), dtype=np.float32)
    for m in range(N_CORES):
        shard = np.asarray(results[m]["out"]).astype(np.float32)
        shard = shard.reshape(C_LOC, TP, 128, KD, W2NB)
        # [c, tp, p, k, col] -> out[tp*W2NB+col, k*128+p, c]
        full[:, :, m * C_LOC:(m + 1) * C_LOC] = (
            shard.transpose(1, 4, 3, 2, 0).reshape(B, DIM, C_LOC))
    return full


def _run(in_maps, trace=False, tmpdir=None):
    if "nc" not in _CACHE:
        _CACHE["nc"] = _build()
    return run_bass_kernel_spmd(
        _CACHE["nc"], in_maps, core_ids=list(range(N_CORES)),
        trace=trace, tmpdir=tmpdir)


def kernel(**inputs):
    in_maps = _prep_in_maps(**inputs)
    res = _run(in_maps)
    return _unshard(res.results)


def kernel_with_profile(tmpdir=None, **inputs):
    """Like kernel() but also returns neuron-profile exec_time_ns."""
    in_maps = _prep_in_maps(**inputs)
    res = _run(in_maps, trace=True, tmpdir=tmpdir)
    return _unshard(res.results), res.exec_time_ns
